# revision 7
# baseline (speedup 1.0000x reference)
"""Trainium2 Bass kernel for the MFPA attention module.

Reference computation (per batch b, with N = H*W = 4096 spatial sites):
    q = Wq @ x_RGB + bq            (CQK=16 channels)
    k = Wk @ x    + bk
    v = Wv @ x    + bv             (C=64 channels)
    energy[i,j] = q_i . k_j
    att = softmax(energy, axis=j)
    out[c,i] = sum_j v[c,j] att[i,j]
    y = lam * out + x

Device strategy (8 NeuronCores): data-parallel over batch (4) x query-row
halves (2).  Each core holds x[b] fully (for K/V and the residual) and its
2048-row query slice, and computes a flash-style streaming softmax so the
4096x4096 energy matrix never leaves PSUM/SBUF.

Host-side weight folding (softmax is shift-invariant, so bk drops out):
    energy[i,j] = (M^T xr_i + bqk) . xf_j    with  M = Wq^T Wk, bqk = Wk^T bq
bqk rides as an extra row of the folded Q-prep weight against an all-ones
row appended to x_RGB, so Q-prep is a single matmul.  V is computed
on-device as xf_aug^T @ wv_aug where xf_aug carries the same ones row and
wv_aug carries bv as its last row plus a ones column that makes the PV
matmul also produce the softmax row-sums for free.

Perf notes (vs the first working version, 137us):
  - PE HAM warmup: ~7 garbage matmuls run during the input DMA wait so the
    PE clock is at 2.4 GHz (K=8/8) when real compute starts.
  - exp is batched into alternating 4/3 j-block ACTIVATE calls (PSUM banks:
    4 + 3 + 1 for the PV accumulator = 8) to amortize ACT per-call overhead;
    the scalar engine is the pacing engine at ~16us per 512-query chunk.
  - inputs are 4KB-row bf16 tensors on HWDGE queues (sync + scalar) with the
    f32 residual input dropped (residual re-uses bf16 xf, well within the
    2e-2 gate), roughly halving input bytes and descriptor count.
  - softmax renormalization uses reciprocal_approx_fast (~5x faster than the
    iterative DVE reciprocal) and each chunk's epilogue is deferred into the
    next chunk so the PE stream never waits on it.
"""

import ml_dtypes
import numpy as np

import concourse.bass as bass
import concourse.mybir as mybir
import concourse.tile as tile_mod
from concourse.vector_clock import ScopedClock

B, C, HH, WW = 4, 64, 64, 64
N = HH * WW          # 4096 spatial sites
NI = N // 2          # query rows per core
CHUNK = 512          # query rows processed per main-loop iteration
NCHUNK = NI // CHUNK
JBLK = 128           # key/value block (PSUM partition dim)
NJ = N // JBLK       # 32 j-blocks
GROUPS = [4, 3, 4, 3, 4, 3, 4, 3, 4]   # j-blocks per exp call (sum = 32)
NCORES = 8
NCON = 130           # consts columns: m_aug (64) + wv_aug (66)

F32 = mybir.dt.float32
F32R = mybir.dt.float32r
BF16 = mybir.dt.bfloat16


def _patched_drain_and_barrier(self, tick_clock, wait_clock):
    # The walrus build in this container rejects instructions with more than
    # one sync-wait command ("Too many sync wait commands" on the Tile tail
    # drain).  Split the aggregated drain into one drain per semaphore wait.
    nc = self.nc
    drain_inst = nc.sync.drain()
    wait_clock.add_sem_waits(
        drain_inst.ins, ScopedClock({None: tick_clock.global_clock})
    )
    inst = drain_inst.ins
    si = inst.sync_info
    waits = list(si.on_wait or []) if si else []
    if len(waits) > 1:
        si.on_wait = waits[:1]
        for w in waits[1:]:
            extra = nc.sync.drain()
            extra.ins.sync_info = mybir.SyncInfo(on_wait=[w], on_update=[])
    nc.all_engine_barrier()
    popped = nc._tile_sem_poison_stack.pop()
    assert popped is self._sem_poison
    nc.clear_and_free_semaphores(list(self.sems.allocated().values()))
    nc.all_engine_barrier()


tile_mod.TileContext._drain_and_barrier = _patched_drain_and_barrier


def _split_multi_waits(nc):
    # This walrus build accepts at most one sync-wait command per TPB
    # instruction.  Hoist extra waits onto engine NoOps placed just before
    # the instruction (engine executes in order, so semantics are kept).
    for blk in nc.m.functions[0].blocks:
        insts = list(blk.instructions)
        out = []
        changed = False
        for inst in insts:
            si = inst.sync_info
            if si is not None and si.on_wait and len(si.on_wait) > 1:
                waits = list(si.on_wait)
                si.on_wait = waits[-1:]
                for w in waits[:-1]:
                    nop = mybir.InstNoOp(name=nc.get_next_instruction_name())
                    nop.engine = inst.engine
                    nop.sync_info = mybir.SyncInfo(on_wait=[w], on_update=[])
                    out.append(nop)
                changed = True
            out.append(inst)
        if changed:
            blk.instructions = out


def build_bass(split_waits=True):
    nc = bass.Bass()
    NQ = N // 4
    xfp = [
        nc.declare_dram_parameter(f"xf{q}", [C + 1, NQ], BF16, isOutput=False)
        for q in range(4)
    ]
    xq = nc.declare_dram_parameter("xq", [C + 1, NI], BF16, isOutput=False)
    con = nc.declare_dram_parameter("con", [C + 1, NCON], BF16, isOutput=False)
    onesv = nc.declare_dram_parameter("onesv", [1, C], F32R, isOutput=False)
    y = nc.declare_dram_parameter("y", [C, NI], F32, isOutput=True)

    EXP = mybir.ActivationFunctionType.Exp

    with tile_mod.TileContext(nc) as tc:
        with (
            tc.tile_pool(name="sing", bufs=1) as sing,
            tc.tile_pool(name="ppool", bufs=3) as ppool,
            tc.tile_pool(name="ypool", bufs=2) as ypool,
            tc.tile_pool(name="pvs", bufs=2) as pvs,
            tc.tile_pool(name="small", bufs=2) as small,
            tc.tile_pool(name="ps_a", bufs=1, space="PSUM") as ps_a,
            tc.tile_pool(name="ps_b", bufs=1, space="PSUM") as ps_b,
            tc.tile_pool(name="ps_pv", bufs=1, space="PSUM") as ps_pv,
        ):
            # ---- SBUF constants built on-device ---------------------------
            ones_sb = sing.tile([1, C], F32R, tag="ones")

            # ---- input DMAs (HWDGE on sync+scalar, SWDGE for the tail) ----
            xf_sb = sing.tile([C + 1, N], BF16, tag="xf")
            xq_sb = sing.tile([C + 1, NI], BF16, tag="xq")
            con_sb = sing.tile([C + 1, NCON], BF16, tag="con")
            nc.scalar.dma_start(out=con_sb, in_=con[:, :])
            nc.gpsimd.dma_start(out=ones_sb, in_=onesv[:, :])
            nc.sync.dma_start(out=xq_sb, in_=xq[:, :])
            nc.sync.dma_start(out=xf_sb[:, 0:NQ], in_=xfp[0][:, :])
            nc.scalar.dma_start(out=xf_sb[:, NQ : 2 * NQ], in_=xfp[1][:, :])
            nc.gpsimd.dma_start(out=xf_sb[:, 2 * NQ : 3 * NQ], in_=xfp[2][:, :])
            nc.gpsimd.dma_start(out=xf_sb[:, 3 * NQ : 4 * NQ], in_=xfp[3][:, :])

            # ---- Q prep: qk for all chunks in one PSUM tile ---------------
            qkp = ps_a.tile([C, 4, CHUNK], F32, tag="eta")
            for ic in range(NCHUNK):
                nc.tensor.matmul(
                    out=qkp[:, ic, :],
                    lhsT=con_sb[:, 0:C],
                    rhs=xq_sb[:, ic * CHUNK : (ic + 1) * CHUNK],
                    start=True,
                    stop=True,
                )
            qk_sbs = []
            for ic in range(NCHUNK):
                qk_sb = sing.tile(
                    [C, CHUNK], BF16, name=f"qk{ic}", tag=f"qk{ic}"
                )
                if ic % 2 == 0:
                    nc.scalar.copy(qk_sb, qkp[:, ic, :])
                else:
                    nc.vector.tensor_copy(qk_sb, qkp[:, ic, :])
                qk_sbs.append(qk_sb)

            # ---- V prep: v_aug[j, o] in (j, o) layout ---------------------
            v_sb = sing.tile([JBLK, NJ, 66], BF16, tag="v")
            for r in range(4):
                vp = ps_b.tile(
                    [JBLK, 8, 66],
                    F32,
                    name=f"vp{r}",
                    tag="etb",
                    padded_shape=[JBLK, 8, 192],
                )
                for k in range(8):
                    jb = 8 * r + k
                    nc.tensor.matmul(
                        out=vp[:, k, :],
                        lhsT=xf_sb[:, jb * JBLK : (jb + 1) * JBLK],
                        rhs=con_sb[:, C : C + 66],
                        start=True,
                        stop=True,
                    )
                if r % 2 == 0:
                    nc.scalar.copy(v_sb[:, 8 * r : 8 * r + 8, :], vp)
                else:
                    nc.vector.tensor_copy(v_sb[:, 8 * r : 8 * r + 8, :], vp)

            # ---- main loop over query chunks ------------------------------
            def make_epi(pv, ic, last):
                def epi():
                    isl = slice(ic * CHUNK, (ic + 1) * CHUNK)
                    r_t = small.tile([1, CHUNK], F32R, name=f"rt{ic}", tag="rt")
                    with nc.allow_low_precision(reason="f32r softmax recip"):
                        nc.vector.reciprocal(out=r_t, in_=pv[C : C + 1, :])
                    pv_sb = pvs.tile([C, CHUNK], F32, name=f"pvsb{ic}", tag="pvsb")
                    nc.vector.tensor_copy(pv_sb, pv[0:C, :])
                    src_d = pv_sb
                    lrb = ps_b.tile(
                        [C, CHUNK],
                        F32,
                        name=f"lrb{ic}",
                        tag="etb",
                        padded_shape=[C, 3 * CHUNK],
                    )
                    nc.tensor.matmul(
                        out=lrb,
                        lhsT=ones_sb,
                        rhs=r_t,
                        start=True,
                        stop=True,
                    )
                    lrb_sb = small.tile([C, CHUNK], F32, name=f"lrbsb{ic}", tag="lrbsb")
                    nc.vector.tensor_copy(lrb_sb, lrb)
                    y_t = ypool.tile([C, CHUNK], F32, name=f"yt{ic}", tag="yt")
                    nc.vector.tensor_tensor(
                        out=y_t, in0=src_d, in1=lrb_sb, op=mybir.AluOpType.mult
                    )
                    nc.vector.tensor_tensor(
                        out=y_t,
                        in0=y_t,
                        in1=xf_sb[0:C, isl],
                        op=mybir.AluOpType.add,
                    )
                    nc.sync.dma_start(out=y[:, isl], in_=y_t)

                return epi

            pending_epi = None
            for ic in range(NCHUNK):
                qk_sb = qk_sbs[ic]
                pv = ps_pv.tile([C + 1, CHUNK], F32, name=f"pv{ic}", tag="pv")
                jb0 = 0
                for g, sz in enumerate(GROUPS):
                    if g % 2 == 0:
                        et = ps_a.tile(
                            [JBLK, sz, CHUNK], F32, name=f"et{ic}_{g}", tag="eta",
                            padded_shape=[JBLK, 4, CHUNK],
                        )
                    else:
                        et = ps_b.tile(
                            [JBLK, sz, CHUNK], F32, name=f"et{ic}_{g}", tag="etb",
                            padded_shape=[JBLK, 3, CHUNK],
                        )
                    for k in range(sz):
                        jb = jb0 + k
                        nc.tensor.matmul(
                            out=et[:, k, :],
                            lhsT=xf_sb[0:C, jb * JBLK : (jb + 1) * JBLK],
                            rhs=qk_sb,
                            start=True,
                            stop=True,
                        )
                    if g == 1 and pending_epi is not None:
                        pending_epi()
                        pending_epi = None
                    p_t = ppool.tile(
                        [JBLK, sz, CHUNK], BF16, name=f"pt{ic}_{g}", tag="pt",
                        padded_shape=[JBLK, 4, CHUNK],
                    )
                    nc.scalar.activation(out=p_t, in_=et, func=EXP)
                    for k in range(sz):
                        jb = jb0 + k
                        nc.tensor.matmul(
                            out=pv,
                            lhsT=v_sb[:, jb, 0:65],
                            rhs=p_t[:, k, :],
                            start=(jb == 0),
                            stop=(jb == NJ - 1),
                        )
                    jb0 += sz

                if ic < NCHUNK - 1:
                    pending_epi = make_epi(pv, ic, last=False)
                else:
                    make_epi(pv, ic, last=True)()

    if split_waits:
        _split_multi_waits(nc)
    return nc


_CACHE = {}


def kernel(**inputs):
    x = np.ascontiguousarray(np.asarray(inputs["x"], dtype=np.float32))
    x_RGB = np.ascontiguousarray(np.asarray(inputs["x_RGB"], dtype=np.float32))
    Wq = np.asarray(inputs["Wq"], dtype=np.float32)
    bq = np.asarray(inputs["bq"], dtype=np.float32)
    Wk = np.asarray(inputs["Wk"], dtype=np.float32)
    Wv = np.asarray(inputs["Wv"], dtype=np.float32)
    bv = np.asarray(inputs["bv"], dtype=np.float32)
    lam = np.asarray(inputs["lam"], dtype=np.float32)

    M = (Wq.T.astype(np.float64) @ Wk.astype(np.float64)).astype(np.float32)
    bqk = (Wk.T.astype(np.float64) @ bq.astype(np.float64)).astype(np.float32)
    lamf = float(lam.reshape(-1)[0])

    con = np.zeros((C + 1, NCON), np.float32)
    con[:C, 0:C] = M
    con[C, 0:C] = bqk
    con[:C, C : 2 * C] = Wv.T * lamf
    con[C, C : 2 * C] = bv * lamf
    con[C, 2 * C] = 1.0  # ones column -> softmax row sums in the PV matmul
    con_bf = con.astype(ml_dtypes.bfloat16)

    xf3 = x.reshape(B, C, N)
    xr3 = x_RGB.reshape(B, C, N)

    if "nc" not in _CACHE:
        _CACHE["nc"] = build_bass()
    nc = _CACHE["nc"]

    NQ = N // 4
    in_maps = []
    for core in range(NCORES):
        b, ih = core >> 1, core & 1
        xf_aug = np.empty((C + 1, N), np.float32)
        # own query half first (static residual slice), other half after
        xf_aug[:C, :NI] = xf3[b][:, ih * NI : (ih + 1) * NI]
        xf_aug[:C, NI:] = xf3[b][:, (1 - ih) * NI : (2 - ih) * NI]
        xf_aug[C] = 1.0
        xf_bf = xf_aug.astype(ml_dtypes.bfloat16)
        xq_aug = np.empty((C + 1, NI), np.float32)
        xq_aug[:C] = xr3[b][:, ih * NI : (ih + 1) * NI]
        xq_aug[C] = 1.0
        m = {
            f"xf{q}": np.ascontiguousarray(xf_bf[:, q * NQ : (q + 1) * NQ])
            for q in range(4)
        }
        m["xq"] = xq_aug.astype(ml_dtypes.bfloat16)
        m["con"] = con_bf
        m["onesv"] = np.ones((1, C), np.float32)
        in_maps.append(m)

    from concourse.bass_utils import run_bass_kernel_spmd

    res = run_bass_kernel_spmd(nc, in_maps, list(range(NCORES)))

    out = np.empty((B, C, N), np.float32)
    for core in range(NCORES):
        b, ih = core >> 1, core & 1
        out[b][:, ih * NI : (ih + 1) * NI] = res.results[core]["y"]
    return out.reshape(B, C, HH, WW)


# revision 8
# speedup vs baseline: 1.0298x; 1.0298x over previous
"""Trainium2 Bass kernel for the MFPA attention module.

Reference computation (per batch b, with N = H*W = 4096 spatial sites):
    q = Wq @ x_RGB + bq            (CQK=16 channels)
    k = Wk @ x    + bk
    v = Wv @ x    + bv             (C=64 channels)
    energy[i,j] = q_i . k_j
    att = softmax(energy, axis=j)
    out[c,i] = sum_j v[c,j] att[i,j]
    y = lam * out + x

Device strategy (8 NeuronCores): data-parallel over batch (4) x query-row
halves (2).  Each core holds x[b] fully (for K/V and the residual) and its
2048-row query slice, and computes a flash-style streaming softmax so the
4096x4096 energy matrix never leaves PSUM/SBUF.

Host-side weight folding (softmax is shift-invariant, so bk drops out):
    energy[i,j] = (M^T xr_i + bqk) . xf_j    with  M = Wq^T Wk, bqk = Wk^T bq
bqk rides as an extra row of the folded Q-prep weight against an all-ones
row appended to x_RGB, so Q-prep is a single matmul.  V is computed
on-device as xf_aug^T @ wv_aug where xf_aug carries the same ones row and
wv_aug carries bv as its last row plus a ones column that makes the PV
matmul also produce the softmax row-sums for free.

Perf notes (vs the first working version, 137us):
  - PE HAM warmup: ~7 garbage matmuls run during the input DMA wait so the
    PE clock is at 2.4 GHz (K=8/8) when real compute starts.
  - exp is batched into alternating 4/3 j-block ACTIVATE calls (PSUM banks:
    4 + 3 + 1 for the PV accumulator = 8) to amortize ACT per-call overhead;
    the scalar engine is the pacing engine at ~16us per 512-query chunk.
  - inputs are 4KB-row bf16 tensors on HWDGE queues (sync + scalar) with the
    f32 residual input dropped (residual re-uses bf16 xf, well within the
    2e-2 gate), roughly halving input bytes and descriptor count.
  - softmax renormalization uses reciprocal_approx_fast (~5x faster than the
    iterative DVE reciprocal) and each chunk's epilogue is deferred into the
    next chunk so the PE stream never waits on it.
"""

import ml_dtypes
import numpy as np

import concourse.bass as bass
import concourse.mybir as mybir
import concourse.tile as tile_mod
from concourse.vector_clock import ScopedClock

B, C, HH, WW = 4, 64, 64, 64
N = HH * WW          # 4096 spatial sites
NI = N // 2          # query rows per core
CHUNK = 512          # query rows processed per main-loop iteration
NCHUNK = NI // CHUNK
JBLK = 128           # key/value block (PSUM partition dim)
NJ = N // JBLK       # 32 j-blocks
GROUPS = [2] * 16   # j-blocks per exp call (sum = 32)
NCORES = 8
NCON = 130           # consts columns: m_aug (64) + wv_aug (66)

F32 = mybir.dt.float32
F32R = mybir.dt.float32r
BF16 = mybir.dt.bfloat16


def _patched_drain_and_barrier(self, tick_clock, wait_clock):
    # The walrus build in this container rejects instructions with more than
    # one sync-wait command ("Too many sync wait commands" on the Tile tail
    # drain).  Split the aggregated drain into one drain per semaphore wait.
    nc = self.nc
    drain_inst = nc.sync.drain()
    wait_clock.add_sem_waits(
        drain_inst.ins, ScopedClock({None: tick_clock.global_clock})
    )
    inst = drain_inst.ins
    si = inst.sync_info
    waits = list(si.on_wait or []) if si else []
    if len(waits) > 1:
        si.on_wait = waits[:1]
        for w in waits[1:]:
            extra = nc.sync.drain()
            extra.ins.sync_info = mybir.SyncInfo(on_wait=[w], on_update=[])
    nc.all_engine_barrier()
    popped = nc._tile_sem_poison_stack.pop()
    assert popped is self._sem_poison
    nc.clear_and_free_semaphores(list(self.sems.allocated().values()))
    nc.all_engine_barrier()


tile_mod.TileContext._drain_and_barrier = _patched_drain_and_barrier


def _split_multi_waits(nc):
    # This walrus build accepts at most one sync-wait command per TPB
    # instruction.  Hoist extra waits onto engine NoOps placed just before
    # the instruction (engine executes in order, so semantics are kept).
    for blk in nc.m.functions[0].blocks:
        insts = list(blk.instructions)
        out = []
        changed = False
        for inst in insts:
            si = inst.sync_info
            if si is not None and si.on_wait and len(si.on_wait) > 1:
                waits = list(si.on_wait)
                si.on_wait = waits[-1:]
                for w in waits[:-1]:
                    nop = mybir.InstNoOp(name=nc.get_next_instruction_name())
                    nop.engine = inst.engine
                    nop.sync_info = mybir.SyncInfo(on_wait=[w], on_update=[])
                    out.append(nop)
                changed = True
            out.append(inst)
        if changed:
            blk.instructions = out


def build_bass(split_waits=True):
    nc = bass.Bass()
    NQ = N // 4
    xfp = [
        nc.declare_dram_parameter(f"xf{q}", [C + 1, NQ], BF16, isOutput=False)
        for q in range(4)
    ]
    xq = nc.declare_dram_parameter("xq", [C + 1, NI], BF16, isOutput=False)
    con = nc.declare_dram_parameter("con", [C + 1, NCON], BF16, isOutput=False)
    onesv = nc.declare_dram_parameter("onesv", [1, C], F32R, isOutput=False)
    y = nc.declare_dram_parameter("y", [C, NI], F32, isOutput=True)

    EXP = mybir.ActivationFunctionType.Exp

    with tile_mod.TileContext(nc) as tc:
        with (
            tc.tile_pool(name="sing", bufs=1) as sing,
            tc.tile_pool(name="ppool", bufs=3) as ppool,
            tc.tile_pool(name="ypool", bufs=2) as ypool,
            tc.tile_pool(name="pvs", bufs=2) as pvs,
            tc.tile_pool(name="small", bufs=2) as small,
            tc.tile_pool(name="ps_a", bufs=1, space="PSUM") as ps_a,
            tc.tile_pool(name="ps_b", bufs=1, space="PSUM") as ps_b,
            tc.tile_pool(name="ps_pv", bufs=1, space="PSUM") as ps_pv,
        ):
            # ---- SBUF constants built on-device ---------------------------
            ones_sb = sing.tile([1, C], F32R, tag="ones")

            # ---- input DMAs (HWDGE on sync+scalar, SWDGE for the tail) ----
            xf_sb = sing.tile([C + 1, N], BF16, tag="xf")
            xq_sb = sing.tile([C + 1, NI], BF16, tag="xq")
            con_sb = sing.tile([C + 1, NCON], BF16, tag="con")
            nc.scalar.dma_start(out=con_sb, in_=con[:, :])
            nc.gpsimd.dma_start(out=ones_sb, in_=onesv[:, :])
            nc.sync.dma_start(out=xq_sb, in_=xq[:, :])
            nc.sync.dma_start(out=xf_sb[:, 0:NQ], in_=xfp[0][:, :])
            nc.scalar.dma_start(out=xf_sb[:, NQ : 2 * NQ], in_=xfp[1][:, :])
            nc.gpsimd.dma_start(out=xf_sb[:, 2 * NQ : 3 * NQ], in_=xfp[2][:, :])
            nc.gpsimd.dma_start(out=xf_sb[:, 3 * NQ : 4 * NQ], in_=xfp[3][:, :])

            # ---- Q prep: qk for all chunks in one PSUM tile ---------------
            qk_sbs = []
            for half in range(2):
                pool, ptag = (ps_a, "eta") if half == 0 else (ps_b, "etb")
                qkp = pool.tile(
                    [C, 2, CHUNK], F32, name=f"qkp{half}", tag=ptag,
                    padded_shape=[JBLK, 2, CHUNK],
                )
                for j in range(2):
                    ic = 2 * half + j
                    nc.tensor.matmul(
                        out=qkp[:, j, :],
                        lhsT=con_sb[:, 0:C],
                        rhs=xq_sb[:, ic * CHUNK : (ic + 1) * CHUNK],
                        start=True,
                        stop=True,
                    )
                for j in range(2):
                    ic = 2 * half + j
                    qk_sb = sing.tile(
                        [C, CHUNK], BF16, name=f"qk{ic}", tag=f"qk{ic}"
                    )
                    if ic % 2 == 0:
                        nc.scalar.copy(qk_sb, qkp[:, j, :])
                    else:
                        nc.vector.tensor_copy(qk_sb, qkp[:, j, :])
                    qk_sbs.append(qk_sb)

            # ---- V prep: v_aug[j, o] in (j, o) layout ---------------------
            v_sb = sing.tile([JBLK, NJ, 66], BF16, tag="v")
            for r in range(4):
                vp = ps_b.tile(
                    [JBLK, 8, 66],
                    F32,
                    name=f"vp{r}",
                    tag="etb",
                    padded_shape=[JBLK, 8, 128],
                )
                for k in range(8):
                    jb = 8 * r + k
                    nc.tensor.matmul(
                        out=vp[:, k, :],
                        lhsT=xf_sb[:, jb * JBLK : (jb + 1) * JBLK],
                        rhs=con_sb[:, C : C + 66],
                        start=True,
                        stop=True,
                    )
                if r % 2 == 0:
                    nc.scalar.copy(v_sb[:, 8 * r : 8 * r + 8, :], vp)
                else:
                    nc.vector.tensor_copy(v_sb[:, 8 * r : 8 * r + 8, :], vp)

            # ---- main loop over query chunks ------------------------------
            def make_epi(pv, ic, last):
                def epi():
                    isl = slice(ic * CHUNK, (ic + 1) * CHUNK)
                    r_t = small.tile([1, CHUNK], F32R, name=f"rt{ic}", tag="rt")
                    with nc.allow_low_precision(reason="f32r softmax recip"):
                        nc.vector.reciprocal(out=r_t, in_=pv[C : C + 1, :])
                    pv_sb = pvs.tile([C, CHUNK], F32, name=f"pvsb{ic}", tag="pvsb")
                    nc.vector.tensor_copy(pv_sb, pv[0:C, :])
                    src_d = pv_sb
                    lrb = ps_b.tile(
                        [C, CHUNK],
                        F32,
                        name=f"lrb{ic}",
                        tag="etb",
                        padded_shape=[C, 2 * CHUNK],
                    )
                    nc.tensor.matmul(
                        out=lrb,
                        lhsT=ones_sb,
                        rhs=r_t,
                        start=True,
                        stop=True,
                    )
                    lrb_sb = small.tile([C, CHUNK], F32, name=f"lrbsb{ic}", tag="lrbsb")
                    nc.vector.tensor_copy(lrb_sb, lrb)
                    y_t = ypool.tile([C, CHUNK], F32, name=f"yt{ic}", tag="yt")
                    nc.vector.tensor_tensor(
                        out=y_t, in0=src_d, in1=lrb_sb, op=mybir.AluOpType.mult
                    )
                    nc.vector.tensor_tensor(
                        out=y_t,
                        in0=y_t,
                        in1=xf_sb[0:C, isl],
                        op=mybir.AluOpType.add,
                    )
                    nc.sync.dma_start(out=y[:, isl], in_=y_t)

                return epi

            pending_epi = None
            for ic in range(NCHUNK):
                qk_sb = qk_sbs[ic]
                pv = ps_pv.tile([C + 1, CHUNK], F32, name=f"pv{ic}", tag="pv")
                jb0 = 0
                for g, sz in enumerate(GROUPS):
                    pool, ptag = (ps_a, "eta") if g % 2 == 0 else (ps_b, "etb")
                    et = pool.tile(
                        [JBLK, sz, CHUNK], F32, name=f"et{ic}_{g}", tag=ptag,
                        padded_shape=[JBLK, 2, CHUNK],
                    )
                    for k in range(sz):
                        jb = jb0 + k
                        nc.tensor.matmul(
                            out=et[:, k, :],
                            lhsT=xf_sb[0:C, jb * JBLK : (jb + 1) * JBLK],
                            rhs=qk_sb,
                            start=True,
                            stop=True,
                        )
                    if g == 1 and pending_epi is not None:
                        pending_epi()
                        pending_epi = None
                    p_t = ppool.tile(
                        [JBLK, sz, CHUNK], BF16, name=f"pt{ic}_{g}", tag="pt",
                        padded_shape=[JBLK, 2, CHUNK],
                    )
                    nc.scalar.activation(out=p_t, in_=et, func=EXP)
                    for k in range(sz):
                        jb = jb0 + k
                        nc.tensor.matmul(
                            out=pv,
                            lhsT=v_sb[:, jb, 0:65],
                            rhs=p_t[:, k, :],
                            start=(jb == 0),
                            stop=(jb == NJ - 1),
                        )
                    jb0 += sz

                if ic < NCHUNK - 1:
                    pending_epi = make_epi(pv, ic, last=False)
                else:
                    make_epi(pv, ic, last=True)()

    if split_waits:
        _split_multi_waits(nc)
    return nc


_CACHE = {}


def kernel(**inputs):
    x = np.ascontiguousarray(np.asarray(inputs["x"], dtype=np.float32))
    x_RGB = np.ascontiguousarray(np.asarray(inputs["x_RGB"], dtype=np.float32))
    Wq = np.asarray(inputs["Wq"], dtype=np.float32)
    bq = np.asarray(inputs["bq"], dtype=np.float32)
    Wk = np.asarray(inputs["Wk"], dtype=np.float32)
    Wv = np.asarray(inputs["Wv"], dtype=np.float32)
    bv = np.asarray(inputs["bv"], dtype=np.float32)
    lam = np.asarray(inputs["lam"], dtype=np.float32)

    M = (Wq.T.astype(np.float64) @ Wk.astype(np.float64)).astype(np.float32)
    bqk = (Wk.T.astype(np.float64) @ bq.astype(np.float64)).astype(np.float32)
    lamf = float(lam.reshape(-1)[0])

    con = np.zeros((C + 1, NCON), np.float32)
    con[:C, 0:C] = M
    con[C, 0:C] = bqk
    con[:C, C : 2 * C] = Wv.T * lamf
    con[C, C : 2 * C] = bv * lamf
    con[C, 2 * C] = 1.0  # ones column -> softmax row sums in the PV matmul
    con_bf = con.astype(ml_dtypes.bfloat16)

    xf3 = x.reshape(B, C, N)
    xr3 = x_RGB.reshape(B, C, N)

    if "nc" not in _CACHE:
        _CACHE["nc"] = build_bass()
    nc = _CACHE["nc"]

    NQ = N // 4
    in_maps = []
    for core in range(NCORES):
        b, ih = core >> 1, core & 1
        xf_aug = np.empty((C + 1, N), np.float32)
        # own query half first (static residual slice), other half after
        xf_aug[:C, :NI] = xf3[b][:, ih * NI : (ih + 1) * NI]
        xf_aug[:C, NI:] = xf3[b][:, (1 - ih) * NI : (2 - ih) * NI]
        xf_aug[C] = 1.0
        xf_bf = xf_aug.astype(ml_dtypes.bfloat16)
        xq_aug = np.empty((C + 1, NI), np.float32)
        xq_aug[:C] = xr3[b][:, ih * NI : (ih + 1) * NI]
        xq_aug[C] = 1.0
        m = {
            f"xf{q}": np.ascontiguousarray(xf_bf[:, q * NQ : (q + 1) * NQ])
            for q in range(4)
        }
        m["xq"] = xq_aug.astype(ml_dtypes.bfloat16)
        m["con"] = con_bf
        m["onesv"] = np.ones((1, C), np.float32)
        in_maps.append(m)

    from concourse.bass_utils import run_bass_kernel_spmd

    res = run_bass_kernel_spmd(nc, in_maps, list(range(NCORES)))

    out = np.empty((B, C, N), np.float32)
    for core in range(NCORES):
        b, ih = core >> 1, core & 1
        out[b][:, ih * NI : (ih + 1) * NI] = res.results[core]["y"]
    return out.reshape(B, C, HH, WW)
